# revision 56
# baseline (speedup 1.0000x reference)
"""Deformable Conv (DCNv2) Trainium2 Bass kernel.

Sharding: 8 cores = 2 batches x 4 H-slabs of 32 output rows each.

Per-core pipeline (single SPMD program, per-core data):
  1. offset/mask 3x3 conv as an 18-step fp32r GEMM on the PE from a
     CHW x-slab resident in SBUF.
  2. PE-transpose offsets to [pixel-partition, (row, k)] layout; compute
     bilinear blend coefficients and gather indices (int16) with wide DVE
     ops.  The quad image has a 2-deep zero pad (131x131 grid) and indices
     are clamped to [-2, 128], so every out-of-range sample reads zeros
     and no validity masking is needed - exact for arbitrary offsets.
     The 4 corner coefficients are written at chunk-16 expanded fp16 width
     (a4e) for the fast stride-0-middle diag build.
  3. dma_gather (SWDGE) pulls, per (kernel-pos k, pixel), one 2KB row of a
     host-built "quad" image (4 bilinear corners x 256 channels, fp16) from
     HBM into [pixel, 4*256] SBUF tiles.  Gathers round-robin across 4
     SWDGE queues; the 4 in-flight transfers aggregate to the ~350 GB/s
     HBM-per-core roofline.
  4. Blend + transpose ride the PE: per (corner, block) the transpose
     matmul uses rhs = diag(a_j) instead of the identity, so corner
     scaling, the 4-corner bilinear sum, and the pixel->channel transpose
     all happen inside one PSUM accumulation group.  DVE only builds the
     diag tiles (identblk x coefficient broadcast, chunk-16 form), keeping
     DVE traffic low - DVE activity stalls SWDGE descriptor generation
     (shared port), so it must stay well under the gather stream.
  5. ACT copies the blended [channel, pixel] PSUM to fp16 SBUF and the
     main GEMM accumulates out[o,p] = sum_{c,k} W[o,c,k] * blended[c,k,p]
     into 4 PSUM banks per quarter.
"""
import os
import numpy as np
import ml_dtypes
from contextlib import ExitStack

import concourse.bass as bass
import concourse.tile as tile
from concourse import bacc, mybir
from concourse.bass_utils import run_bass_kernel_spmd
from concourse.masks import make_identity
from concourse import library_config

F32 = mybir.dt.float32
F32R = mybir.dt.float32r
BF16 = mybir.dt.bfloat16
F16 = mybir.dt.float16
I16 = mybir.dt.int16
AF = mybir.ActivationFunctionType
OP = mybir.AluOpType

CIN = 256
COUT = 256
K2 = 9
H = W = 128
B = 2
HS = 32           # output rows per core
NCORE = 8
QD = H + 3        # quad grid dim (131): 2-pad so clamped indices hit zeros
NQ = QD * QD      # 17161 quad rows
MAGIC = 12582912.0  # 1.5 * 2**23 fp32 round-to-int magic

NBLK = 32         # pixel blocks (rows) per core
NQRT = 4          # quarters (8 rows each) per core
BPQ = 8           # blocks per quarter


# ----------------------------------------------------------------------------
# device program
# ----------------------------------------------------------------------------

def build_program():
    nc = bacc.Bacc("TRN2", target_bir_lowering=False, debug=False,
                   num_swdge_queues=4)

    xslab = nc.dram_tensor("xslab", [2, 128, 34, 130], BF16, kind="ExternalInput")
    quad = nc.dram_tensor("quad", [NQ, 1024], F16, kind="ExternalInput")
    womt = nc.dram_tensor("womt", [18, 128, 32], BF16, kind="ExternalInput")
    wmaint = nc.dram_tensor("wmaint", [128, 36, 128], F16, kind="ExternalInput")
    baseY = nc.dram_tensor("baseY", [128, K2, NBLK], F32, kind="ExternalInput")
    baseX = nc.dram_tensor("baseX", [128, K2, NBLK], F32, kind="ExternalInput")
    bofft = nc.dram_tensor("bofft", [32, 1], F32, kind="ExternalInput")
    out = nc.dram_tensor("out", [2, 128, HS, W], F32, kind="ExternalOutput")

    with tile.TileContext(nc) as tc, ExitStack() as ctx:
        const = ctx.enter_context(tc.tile_pool(name="const", bufs=1))
        work = ctx.enter_context(tc.tile_pool(name="work", bufs=1))
        coeff = ctx.enter_context(tc.tile_pool(name="coeff", bufs=1))
        tmp = ctx.enter_context(tc.tile_pool(name="tmp", bufs=4))
        gpool = ctx.enter_context(tc.tile_pool(name="gpool", bufs=5))
        bpool = ctx.enter_context(tc.tile_pool(name="bpool", bufs=2))
        rhsp = ctx.enter_context(tc.tile_pool(name="rhsp", bufs=3))
        outp = ctx.enter_context(tc.tile_pool(name="outp", bufs=3))
        psB = ctx.enter_context(tc.tile_pool(name="psB", bufs=2, space="PSUM"))
        psC = ctx.enter_context(tc.tile_pool(name="psC", bufs=2, space="PSUM"))
        psO = ctx.enter_context(tc.tile_pool(name="psO", bufs=1, space="PSUM"))

        # ---- constants -----------------------------------------------------
        nc.gpsimd.load_library(library_config.mlp)
        ident16 = const.tile([128, 128], F16)
        make_identity(nc, ident16[:])
        identf = const.tile([128, 128], F32)
        make_identity(nc, identf[:])
        identblk = const.tile([128, BPQ, 128], F16)
        for bl in range(BPQ):
            nc.scalar.copy(identblk[:, bl, :], ident16[:])

        wom_sb = const.tile([128, 18, 32], BF16)
        nc.sync.dma_start(wom_sb[:], womt[:].rearrange("t c o -> c t o"))
        xs = []
        for ch in range(2):
            t = work.tile([128, 34, 130], BF16, tag=f"xs{ch}")
            nc.sync.dma_start(t[:, 0:11], xslab[ch][:, 0:11])
            xs.append(t)
        bY = const.tile([128, K2, NBLK], F32)
        nc.sync.dma_start(bY[:], baseY[:])
        bX = const.tile([128, K2, NBLK], F32)
        nc.sync.dma_start(bX[:], baseX[:])
        bo = const.tile([32, 1], F32)
        nc.sync.dma_start(bo[:], bofft[:])
        # big background loads ride the ACT HWDGE ring so quarter-0's idx
        # fold DMAs (SP ring) aren't queued behind them
        for ch in range(2):
            nc.scalar.dma_start(xs[ch][:, 11:34], xslab[ch][:, 11:34])
        wm_sb = const.tile([128, 36, 128], F16)
        nc.scalar.dma_start(wm_sb[:], wmaint[:])

        # broadcast-constant columns: [MAGIC, -MAGIC, 0, -2, 128, 264]
        # clamp range [-2, 128]: both corners of a clamped sample land on
        # quad zero-pad rows, so out-of-range samples are exactly 0 and no
        # validity masking is needed.  264 = 2*QD + 2 (grid offset).
        cst = const.tile([128, 8], F32)
        for i, v in enumerate((MAGIC, -MAGIC, 0.0, -2.0, float(H),
                               float(2 * QD + 2))):
            nc.vector.memset(cst[:, i:i + 1], v)

        def cb(i):
            return cst[:, i:i + 1, None].broadcast_to([128, K2, 8])

        zi16 = const.tile([16, 1], I16)
        nc.vector.memset(zi16[:], 0)

        # ---- per-quarter pipeline; emit_prep is a generator whose chunks
        # are interleaved between the main loop's k-iterations so prep work
        # never bursts into the engine queues.
        def emit_prep_om(q):
            # 1. offset/mask conv for this quarter (8 rows, 2 N-blocks)
            sb_om = work.tile([32, 8 * W], F32, tag="sb_om", name="sb_om",
                              bufs=3)
            for lnb in range(2):
                nb = q * 2 + lnb
                ps = psC.tile([32, 512], F32, tag="omstage", name="ps_om")
                for t in range(18):
                    k, ch = divmod(t, 2)
                    ky, kx = divmod(k, 3)
                    rhs = xs[ch][:, nb * 4 + ky:nb * 4 + ky + 4, kx:kx + 128]
                    nc.tensor.matmul(
                        ps[:],
                        wom_sb[:, t, :],
                        rhs,
                        start=(t == 0),
                        stop=(t == 17),
                    )
                nc.scalar.activation(sb_om[:, lnb * 512:(lnb + 1) * 512],
                                     ps[:], AF.Identity, bias=bo[:])

            # 2a. transpose offsets to [pix, (blk, ch27)]
            t_off = coeff.tile([128, 27, 8], F32, tag="t_off", name="t_off",
                               bufs=3)
            for g in range(2):
                tp = psC.tile([128, 128], F32, tag="omstage", name="tp_o")
                for j in range(4):
                    bl = g * 4 + j
                    nc.tensor.transpose(
                        tp[:, j * 27:(j + 1) * 27],
                        sb_om[0:27, bl * 128:(bl + 1) * 128],
                        identf[0:27, 0:27],
                    )
                nc.scalar.copy(t_off[:, :, g * 4:(g + 1) * 4]
                               .rearrange('p c b -> p b c'), tp[:, 0:108])
            return t_off

        def emit_prep_coeff(q, t_off):
            # 2b. coefficient + index pipeline (wide [128, 9, 8] ops)
            dy = t_off[:, 0:9, :]
            dx = t_off[:, 9:18, :]
            ml = t_off[:, 18:27, :]
            bYq = bY[:, :, q * 8:(q + 1) * 8]
            bXq = bX[:, :, q * 8:(q + 1) * 8]

            def ctile(tag):
                return coeff.tile([128, K2, 8], F32, tag=tag, name=tag,
                                  bufs=3)

            m = ctile('m')
            nc.scalar.activation(m[:], ml, AF.Sigmoid)

            pyp = ctile('pyp')
            nc.vector.tensor_add(pyp[:], dy, bYq)
            y0 = ctile('y0')
            nc.vector.tensor_tensor(y0[:], pyp[:], cb(0), OP.add)
            nc.vector.tensor_tensor(y0[:], y0[:], cb(1), OP.add)
            wy = ctile('wy')
            nc.vector.scalar_tensor_tensor(wy[:], pyp[:], 0.5, y0[:], OP.add,
                                           OP.subtract)
            pxp = ctile('pxp')
            nc.vector.tensor_add(pxp[:], dx, bXq)
            x0 = ctile('x0')
            nc.vector.tensor_tensor(x0[:], pxp[:], cb(0), OP.add)
            nc.vector.tensor_tensor(x0[:], x0[:], cb(1), OP.add)
            wx = ctile('wx')
            nc.vector.scalar_tensor_tensor(wx[:], pxp[:], 0.5, x0[:], OP.add,
                                           OP.subtract)

            y0c = ctile('y0c')
            nc.vector.tensor_tensor(y0c[:], y0[:], cb(3), OP.max)
            nc.vector.tensor_tensor(y0c[:], y0c[:], cb(4), OP.min)
            x0c = ctile('x0c')
            nc.vector.tensor_tensor(x0c[:], x0[:], cb(3), OP.max)
            nc.vector.tensor_tensor(x0c[:], x0c[:], cb(4), OP.min)

            idxf = ctile('idxf')
            nc.vector.scalar_tensor_tensor(idxf[:], y0c[:], float(QD), x0c[:],
                                           OP.mult, OP.add)
            nc.vector.tensor_tensor(idxf[:], idxf[:], cb(5), OP.add)
            idx16 = coeff.tile([128, K2, 8], I16, tag="idx16", name="idx16",
                               bufs=3)
            nc.vector.tensor_tensor(idx16[:], idxf[:], cb(2), OP.add)

            # 2c. fold idx to gather layout [16, (k, blk, g)] + replicate.
            # Two hops: 8 fully-contiguous partition-fold DMAs into
            # [16, g, k, blk], then one lock-free DVE bypass-copy to
            # transpose the free dims to [16, k, blk, g].  The gather
            # ucode (queue 0) reads idxs from partitions 0-31 only, so
            # replicate just that far.
            idxt = coeff.tile([16, 8, K2, 8], I16, tag="idxt", name="idxt",
                              bufs=3)
            for g in range(8):
                srcv = idx16[g * 16:(g + 1) * 16, :, :]
                nc.sync.dma_start(idxt[:, g], srcv)
            idxg = coeff.tile([128, K2, 8, 8], I16, tag="idxg", name="idxg",
                              bufs=3)
            nc.vector.tensor_tensor(
                idxg[0:16], idxt[:].rearrange('q g k b -> q k b g'),
                zi16[:, :, None, None].broadcast_to([16, K2, 8, 8]),
                OP.add)
            nc.sync.dma_start(idxg[16:32], idxg[0:16])
            nc.sync.dma_start(idxg[32:64], idxg[0:32])
            nc.sync.dma_start(idxg[64:128], idxg[0:64])

            # bilinear products (validity rides the quad zero-pad):
            # g1 = m*wy, g0 = m*(1-wy); a01 = g0*wx, a00 = g0-a01,
            # a11 = g1*wx, a10 = g1-a11.  The 4 products are written
            # directly at chunk-16 expanded width (fp16) for the fast
            # D4-form diag build in emit_main.
            g1 = ctile('g1')
            nc.vector.tensor_mul(g1[:], m[:], wy[:])
            g0 = ctile('g0')
            nc.vector.tensor_sub(g0[:], m[:], g1[:])

            a4e = coeff.tile([128, 4, K2, 8, 16], F16, tag="a4e",
                             name="a4e", bufs=2)

            def b16(t):
                return t[:, :, :, None].broadcast_to([128, K2, 8, 16])

            nc.vector.tensor_tensor(a4e[:, 1], b16(g0), b16(wx), OP.mult)
            nc.vector.tensor_tensor(a4e[:, 0], b16(g0), a4e[:, 1],
                                    OP.subtract)
            nc.vector.tensor_tensor(a4e[:, 3], b16(g1), b16(wx), OP.mult)
            nc.vector.tensor_tensor(a4e[:, 2], b16(g1), a4e[:, 3],
                                    OP.subtract)

            return a4e, idxg

        def emit_main(q, coefs, mid=None):
            a4e, idxg = coefs
            midc = None
            # 3-5. gather / diag-scale-transpose+sum (PE) / GEMM
            po = [psO.tile([128, 512], F32, tag=f"po{i}", name=f"po{i}")
                  for i in range(4)]
            for k in range(K2):
                gbuf = gpool.tile([128, BPQ, 1024], F16, tag="gbuf")
                nc.gpsimd.dma_gather(
                    gbuf[:],
                    quad[:],
                    idxg[:, k, :, :],
                    num_idxs=BPQ * 128,
                    num_idxs_reg=BPQ * 128,
                    elem_size=1024,
                    single_packet=False,
                    queue_num=(q * K2 + k) % 4,
                )
                # The per-(pixel,k) corner coefficients ride the PE: the
                # transpose matmuls use rhs = diag(a_j) per (corner, block)
                # instead of the identity, so scale + 4-corner sum + pixel
                # transpose all happen in the PSUM accumulation.  DVE only
                # builds the diag tiles (ident x per-partition coeff).
                dg = bpool.tile([128, 4, BPQ, 128], F16, tag="diag",
                                name="diag", bufs=3)
                for j in range(4):
                    nc.vector.tensor_tensor(
                        dg[:, j].rearrange('p b (r c) -> p b r c', r=8),
                        identblk[:].rearrange('p b (r c) -> p b r c', r=8),
                        a4e[:, j, k, :, None, :].broadcast_to(
                            [128, BPQ, 8, 16]),
                        OP.mult)
                if k == 2 and mid is not None:
                    midc = mid()

                for j2 in range(2):
                    for ct in range(2):
                        tp = psB.tile([128, 512], F32, tag="stage",
                                      name="tp_b")
                        for r in range(4):
                            bl = j2 * 4 + r
                            for j in range(4):
                                nc.tensor.matmul(
                                    tp[:, r * 128:(r + 1) * 128],
                                    gbuf[:, bl,
                                         j * 256 + ct * 128:
                                         j * 256 + ct * 128 + 128],
                                    dg[:, j, bl, :],
                                    start=(j == 0),
                                    stop=(j == 3),
                                )
                        rhs16 = rhsp.tile([128, 512], F16, tag="rhs",
                                          name="rhs")
                        nc.scalar.copy(rhs16[:], tp[:])
                        for ot in range(2):
                            widx = (k * 2 + ct) * 2 + ot
                            nc.tensor.matmul(
                                po[j2 * 2 + ot][:],
                                wm_sb[:, widx, :],
                                rhs16[:],
                                start=(k == 0 and ct == 0),
                                stop=(k == 8 and ct == 1),
                            )
            for j2 in range(2):
                og = q * 2 + j2
                for ot in range(2):
                    o_sb = outp.tile([128, 4, 128], F32, tag="osb")
                    nc.scalar.copy(o_sb[:], po[j2 * 2 + ot][:])
                    nc.sync.dma_start(out[ot, :, og * 4:(og + 1) * 4, :], o_sb[:])
            return midc

        # om(q+1) fills PE slack before/between mains; the coeff pipeline
        # of q+1 is emitted mid-main(q) (after k=2's diag) so its indices
        # are ready well before the boundary and the q+1 gathers never
        # wait on DVE.
        t0 = emit_prep_om(0)
        c0 = emit_prep_coeff(0, t0)
        t1 = emit_prep_om(1)
        c1 = emit_main(0, c0, mid=lambda: emit_prep_coeff(1, t1))
        t2 = emit_prep_om(2)
        c2 = emit_main(1, c1, mid=lambda: emit_prep_coeff(2, t2))
        t3 = emit_prep_om(3)
        c3 = emit_main(2, c2, mid=lambda: emit_prep_coeff(3, t3))
        emit_main(3, c3)

    nc.finalize()
    return nc


# ----------------------------------------------------------------------------
# host-side data prep
# ----------------------------------------------------------------------------

def build_in_maps(x, w_conv, b_conv, w_off, b_off, w_mask, b_mask):
    x = np.ascontiguousarray(x, np.float32)

    # quad image per batch: quad[(y0+2)*131+(x0+2), (j,c)] fp16, 2-pad so
    # clamped out-of-range corners read guaranteed zeros
    quads = []
    for b in range(B):
        xp = np.zeros((H + 4, W + 4, CIN), np.float32)
        xp[2:-2, 2:-2] = x[b].transpose(1, 2, 0)
        q = np.empty((QD, QD, 4, CIN), np.float16)
        q[:, :, 0] = xp[0:QD, 0:QD]
        q[:, :, 1] = xp[0:QD, 1:QD + 1]
        q[:, :, 2] = xp[1:QD + 1, 0:QD]
        q[:, :, 3] = xp[1:QD + 1, 1:QD + 1]
        quads.append(np.ascontiguousarray(q.reshape(NQ, 1024)))

    # offset/mask weights, output channels reordered to [dy*9, dx*9, ml*9]
    wom = np.concatenate([w_off, w_mask], 0).reshape(27, CIN, K2)  # [o,c,k]
    perm = np.concatenate([np.arange(0, 18, 2), np.arange(1, 18, 2),
                           np.arange(18, 27)])
    womp = wom[perm]                                   # [27(dy,dx,ml), c, k]
    womt = np.zeros((18, 128, 32), np.float32)
    for t in range(18):
        k, ch = divmod(t, 2)
        womt[t, :, 0:27] = womp[:, ch * 128:(ch + 1) * 128, k].T
    bom = np.concatenate([b_off, b_mask]).astype(np.float32)[perm]
    bofft = np.zeros((32, 1), np.float32)
    bofft[0:27, 0] = bom

    # main weights [c, (k,ct,ot), o] fp16
    wc = w_conv.reshape(COUT, CIN, K2)
    wmaint = np.zeros((128, 36, 128), np.float16)
    for k in range(K2):
        for ct in range(2):
            for ot in range(2):
                widx = (k * 2 + ct) * 2 + ot
                wmaint[:, widx, :] = (
                    wc[ot * 128:(ot + 1) * 128, ct * 128:(ct + 1) * 128, k].T
                )

    ky = (np.arange(K2) // 3).astype(np.float32)
    kx = (np.arange(K2) % 3).astype(np.float32)
    bXc = np.zeros((128, K2, NBLK), np.float32)
    bXc[:] = (np.arange(128, dtype=np.float32)[:, None, None]
              + kx[None, :, None] - 1.5)

    in_maps = []
    for core in range(NCORE):
        b, slab = divmod(core, 4)
        h0 = slab * HS
        xsl = np.zeros((2, 128, 34, 130), np.float32)
        r_lo = max(0, h0 - 1)
        r_hi = min(H, h0 + HS + 1)
        xsl[:, :, (r_lo - (h0 - 1)):(r_hi - (h0 - 1)), 1:129] = (
            x[b].reshape(2, 128, H, W)[:, :, r_lo:r_hi, :]
        )
        bYc = np.zeros((128, K2, NBLK), np.float32)
        bYc[:] = ((h0 + np.arange(NBLK, dtype=np.float32))[None, None, :]
                  + ky[None, :, None] - 1.5)
        in_maps.append({
"xslab": xsl.astype(ml_dtypes.bfloat16),
            "quad": quads[b],
            "womt": womt.astype(ml_dtypes.bfloat16),
            "wmaint": wmaint,
            "baseY": bYc,
            "baseX": np.ascontiguousarray(bXc),
            "bofft": bofft,
        })
    return in_maps


_PROGRAM = None
LAST_EXEC_NS = None
LAST_RESULTS = None


def kernel(x, w_conv, b_conv, w_off, b_off, w_mask, b_mask):
    global _PROGRAM, LAST_EXEC_NS, LAST_RESULTS
    in_maps = build_in_maps(x, w_conv, b_conv, w_off, b_off, w_mask, b_mask)
    if _PROGRAM is None:
        _PROGRAM = build_program()
    nc = _PROGRAM
    trace = bool(os.environ.get("DCN_TRACE"))
    res = run_bass_kernel_spmd(nc, in_maps, core_ids=list(range(NCORE)),
                               trace=trace)
    LAST_EXEC_NS = res.exec_time_ns
    LAST_RESULTS = res
    out = np.zeros((B, COUT, H, W), np.float32)
    for core in range(NCORE):
        b, slab = divmod(core, 4)
        h0 = slab * HS
        oc = res.results[core]["out"]  # [2, 128, 32, 128]
        out[b, 0:128, h0:h0 + HS, :] = oc[0]
        out[b, 128:256, h0:h0 + HS, :] = oc[1]
    # b_conv is zeros in the reference setup, but add anyway for correctness
    out += np.asarray(b_conv, np.float32)[None, :, None, None]
    return out



# revision 57
# speedup vs baseline: 1.0414x; 1.0414x over previous
"""Deformable Conv (DCNv2) Trainium2 Bass kernel.

Sharding: 8 cores = 2 batches x 4 H-slabs of 32 output rows each.

Per-core pipeline (single SPMD program, per-core data):
  1. offset/mask 3x3 conv as an 18-step fp32r GEMM on the PE from a
     CHW x-slab resident in SBUF.
  2. PE-transpose offsets to [pixel-partition, (row, k)] layout; compute
     bilinear blend coefficients and gather indices (int16) with wide DVE
     ops.  The quad image has a 2-deep zero pad (131x131 grid) and indices
     are clamped to [-2, 128], so every out-of-range sample reads zeros
     and no validity masking is needed - exact for arbitrary offsets.
     The 4 corner coefficients are written at chunk-16 expanded fp16 width
     (a4e) for the fast stride-0-middle diag build.
  3. dma_gather (SWDGE) pulls, per (kernel-pos k, pixel), one 2KB row of a
     host-built "quad" image (4 bilinear corners x 256 channels, fp16) from
     HBM into [pixel, 4*256] SBUF tiles.  Gathers round-robin across 4
     SWDGE queues; the 4 in-flight transfers aggregate to the ~350 GB/s
     HBM-per-core roofline.
  4. Blend + transpose ride the PE: per (corner, block) the transpose
     matmul uses rhs = diag(a_j) instead of the identity, so corner
     scaling, the 4-corner bilinear sum, and the pixel->channel transpose
     all happen inside one PSUM accumulation group.  DVE only builds the
     diag tiles (identblk x coefficient broadcast, chunk-16 form), keeping
     DVE traffic low - DVE activity stalls SWDGE descriptor generation
     (shared port), so it must stay well under the gather stream.
  5. ACT copies the blended [channel, pixel] PSUM to fp16 SBUF and the
     main GEMM accumulates out[o,p] = sum_{c,k} W[o,c,k] * blended[c,k,p]
     into 4 PSUM banks per quarter.
"""
import os
import numpy as np
import ml_dtypes
from contextlib import ExitStack

import concourse.bass as bass
import concourse.tile as tile
from concourse import bacc, mybir
from concourse.bass_utils import run_bass_kernel_spmd
from concourse.masks import make_identity
from concourse import library_config

F32 = mybir.dt.float32
F32R = mybir.dt.float32r
BF16 = mybir.dt.bfloat16
F16 = mybir.dt.float16
I16 = mybir.dt.int16
AF = mybir.ActivationFunctionType
OP = mybir.AluOpType

CIN = 256
COUT = 256
K2 = 9
H = W = 128
B = 2
HS = 32           # output rows per core
NCORE = 8
QD = H + 3        # quad grid dim (131): 2-pad so clamped indices hit zeros
NQ = QD * QD      # 17161 quad rows
MAGIC = 12582912.0  # 1.5 * 2**23 fp32 round-to-int magic

NBLK = 32         # pixel blocks (rows) per core
NQRT = 4          # quarters (8 rows each) per core
BPQ = 8           # blocks per quarter


# ----------------------------------------------------------------------------
# device program
# ----------------------------------------------------------------------------

def build_program():
    nc = bacc.Bacc("TRN2", target_bir_lowering=False, debug=False,
                   num_swdge_queues=4)

    xslab = nc.dram_tensor("xslab", [2, 128, 34, 130], F32R, kind="ExternalInput")
    quad = nc.dram_tensor("quad", [NQ, 1024], F16, kind="ExternalInput")
    womt = nc.dram_tensor("womt", [18, 128, 32], F32R, kind="ExternalInput")
    wmaint = nc.dram_tensor("wmaint", [128, 36, 128], F16, kind="ExternalInput")
    baseY = nc.dram_tensor("baseY", [128, K2, NBLK], F32, kind="ExternalInput")
    baseX = nc.dram_tensor("baseX", [128, K2, NBLK], F32, kind="ExternalInput")
    bofft = nc.dram_tensor("bofft", [32, 1], F32, kind="ExternalInput")
    out = nc.dram_tensor("out", [2, 128, HS, W], F32, kind="ExternalOutput")

    with tile.TileContext(nc) as tc, ExitStack() as ctx:
        const = ctx.enter_context(tc.tile_pool(name="const", bufs=1))
        work = ctx.enter_context(tc.tile_pool(name="work", bufs=1))
        coeff = ctx.enter_context(tc.tile_pool(name="coeff", bufs=1))
        tmp = ctx.enter_context(tc.tile_pool(name="tmp", bufs=4))
        gpool = ctx.enter_context(tc.tile_pool(name="gpool", bufs=4))
        bpool = ctx.enter_context(tc.tile_pool(name="bpool", bufs=2))
        rhsp = ctx.enter_context(tc.tile_pool(name="rhsp", bufs=3))
        outp = ctx.enter_context(tc.tile_pool(name="outp", bufs=3))
        psB = ctx.enter_context(tc.tile_pool(name="psB", bufs=2, space="PSUM"))
        psC = ctx.enter_context(tc.tile_pool(name="psC", bufs=2, space="PSUM"))
        psO = ctx.enter_context(tc.tile_pool(name="psO", bufs=1, space="PSUM"))

        # ---- constants -----------------------------------------------------
        nc.gpsimd.load_library(library_config.mlp)
        ident16 = const.tile([128, 128], F16)
        make_identity(nc, ident16[:])
        identf = const.tile([128, 128], F32)
        make_identity(nc, identf[:])
        identblk = const.tile([128, BPQ, 128], F16)
        for bl in range(BPQ):
            nc.scalar.copy(identblk[:, bl, :], ident16[:])

        wom_sb = const.tile([128, 18, 32], F32R)
        nc.sync.dma_start(wom_sb[:], womt[:].rearrange("t c o -> c t o"))
        xs = []
        for ch in range(2):
            t = work.tile([128, 34, 130], F32R, tag=f"xs{ch}")
            nc.sync.dma_start(t[:, 0:11], xslab[ch][:, 0:11])
            xs.append(t)
        bY = const.tile([128, K2, NBLK], F32)
        nc.sync.dma_start(bY[:], baseY[:])
        bX = const.tile([128, K2, NBLK], F32)
        nc.sync.dma_start(bX[:], baseX[:])
        bo = const.tile([32, 1], F32)
        nc.sync.dma_start(bo[:], bofft[:])
        # big background loads ride the ACT HWDGE ring so quarter-0's idx
        # fold DMAs (SP ring) aren't queued behind them
        for ch in range(2):
            nc.scalar.dma_start(xs[ch][:, 11:34], xslab[ch][:, 11:34])
        wm_sb = const.tile([128, 36, 128], F16)
        nc.scalar.dma_start(wm_sb[:], wmaint[:])

        # broadcast-constant columns: [MAGIC, -MAGIC, 0, -2, 128, 264]
        # clamp range [-2, 128]: both corners of a clamped sample land on
        # quad zero-pad rows, so out-of-range samples are exactly 0 and no
        # validity masking is needed.  264 = 2*QD + 2 (grid offset).
        cst = const.tile([128, 8], F32)
        for i, v in enumerate((MAGIC, -MAGIC, 0.0, -2.0, float(H),
                               float(2 * QD + 2))):
            nc.vector.memset(cst[:, i:i + 1], v)

        def cb(i):
            return cst[:, i:i + 1, None].broadcast_to([128, K2, 8])

        zi16 = const.tile([16, 1], I16)
        nc.vector.memset(zi16[:], 0)

        # ---- per-quarter pipeline; emit_prep is a generator whose chunks
        # are interleaved between the main loop's k-iterations so prep work
        # never bursts into the engine queues.
        def emit_prep_om(q):
            # 1. offset/mask conv for this quarter (8 rows, 2 N-blocks)
            sb_om = work.tile([32, 8 * W], F32, tag="sb_om", name="sb_om",
                              bufs=3)
            for lnb in range(2):
                nb = q * 2 + lnb
                ps = psC.tile([32, 512], F32, tag="omstage", name="ps_om")
                for t in range(18):
                    k, ch = divmod(t, 2)
                    ky, kx = divmod(k, 3)
                    rhs = xs[ch][:, nb * 4 + ky:nb * 4 + ky + 4, kx:kx + 128]
                    nc.tensor.matmul(
                        ps[:],
                        wom_sb[:, t, :],
                        rhs,
                        start=(t == 0),
                        stop=(t == 17),
                    )
                nc.scalar.activation(sb_om[:, lnb * 512:(lnb + 1) * 512],
                                     ps[:], AF.Identity, bias=bo[:])

            # 2a. transpose offsets to [pix, (blk, ch27)]
            t_off = coeff.tile([128, 27, 8], F32, tag="t_off", name="t_off",
                               bufs=3)
            for g in range(2):
                tp = psC.tile([128, 128], F32, tag="omstage", name="tp_o")
                for j in range(4):
                    bl = g * 4 + j
                    nc.tensor.transpose(
                        tp[:, j * 27:(j + 1) * 27],
                        sb_om[0:27, bl * 128:(bl + 1) * 128],
                        identf[0:27, 0:27],
                    )
                nc.scalar.copy(t_off[:, :, g * 4:(g + 1) * 4]
                               .rearrange('p c b -> p b c'), tp[:, 0:108])
            return t_off

        def emit_prep_coeff(q, t_off):
            # 2b. coefficient + index pipeline (wide [128, 9, 8] ops)
            dy = t_off[:, 0:9, :]
            dx = t_off[:, 9:18, :]
            ml = t_off[:, 18:27, :]
            bYq = bY[:, :, q * 8:(q + 1) * 8]
            bXq = bX[:, :, q * 8:(q + 1) * 8]

            def ctile(tag):
                return coeff.tile([128, K2, 8], F32, tag=tag, name=tag,
                                  bufs=3)

            m = ctile('m')
            nc.scalar.activation(m[:], ml, AF.Sigmoid)

            pyp = ctile('pyp')
            nc.vector.tensor_add(pyp[:], dy, bYq)
            y0 = ctile('y0')
            nc.vector.tensor_tensor(y0[:], pyp[:], cb(0), OP.add)
            nc.vector.tensor_tensor(y0[:], y0[:], cb(1), OP.add)
            wy = ctile('wy')
            nc.vector.scalar_tensor_tensor(wy[:], pyp[:], 0.5, y0[:], OP.add,
                                           OP.subtract)
            pxp = ctile('pxp')
            nc.vector.tensor_add(pxp[:], dx, bXq)
            x0 = ctile('x0')
            nc.vector.tensor_tensor(x0[:], pxp[:], cb(0), OP.add)
            nc.vector.tensor_tensor(x0[:], x0[:], cb(1), OP.add)
            wx = ctile('wx')
            nc.vector.scalar_tensor_tensor(wx[:], pxp[:], 0.5, x0[:], OP.add,
                                           OP.subtract)

            y0c = ctile('y0c')
            nc.vector.tensor_tensor(y0c[:], y0[:], cb(3), OP.max)
            nc.vector.tensor_tensor(y0c[:], y0c[:], cb(4), OP.min)
            x0c = ctile('x0c')
            nc.vector.tensor_tensor(x0c[:], x0[:], cb(3), OP.max)
            nc.vector.tensor_tensor(x0c[:], x0c[:], cb(4), OP.min)

            idxf = ctile('idxf')
            nc.vector.scalar_tensor_tensor(idxf[:], y0c[:], float(QD), x0c[:],
                                           OP.mult, OP.add)
            nc.vector.tensor_tensor(idxf[:], idxf[:], cb(5), OP.add)
            idx16 = coeff.tile([128, K2, 8], I16, tag="idx16", name="idx16",
                               bufs=3)
            nc.vector.tensor_tensor(idx16[:], idxf[:], cb(2), OP.add)

            # 2c. fold idx to gather layout [16, (k, blk, g)] + replicate.
            # Two hops: 8 fully-contiguous partition-fold DMAs into
            # [16, g, k, blk], then one lock-free DVE bypass-copy to
            # transpose the free dims to [16, k, blk, g].  The gather
            # ucode (queue 0) reads idxs from partitions 0-31 only, so
            # replicate just that far.
            idxt = coeff.tile([16, 8, K2, 8], I16, tag="idxt", name="idxt",
                              bufs=3)
            for g in range(8):
                srcv = idx16[g * 16:(g + 1) * 16, :, :]
                nc.sync.dma_start(idxt[:, g], srcv)
            idxg = coeff.tile([128, K2, 8, 8], I16, tag="idxg", name="idxg",
                              bufs=3)
            nc.vector.tensor_tensor(
                idxg[0:16], idxt[:].rearrange('q g k b -> q k b g'),
                zi16[:, :, None, None].broadcast_to([16, K2, 8, 8]),
                OP.add)
            nc.sync.dma_start(idxg[16:32], idxg[0:16])
            nc.sync.dma_start(idxg[32:64], idxg[0:32])
            nc.sync.dma_start(idxg[64:128], idxg[0:64])

            # bilinear products (validity rides the quad zero-pad):
            # g1 = m*wy, g0 = m*(1-wy); a01 = g0*wx, a00 = g0-a01,
            # a11 = g1*wx, a10 = g1-a11.  The 4 products are written
            # directly at chunk-16 expanded width (fp16) for the fast
            # D4-form diag build in emit_main.
            g1 = ctile('g1')
            nc.vector.tensor_mul(g1[:], m[:], wy[:])
            g0 = ctile('g0')
            nc.vector.tensor_sub(g0[:], m[:], g1[:])

            a4e = coeff.tile([128, 4, K2, 8, 16], F16, tag="a4e",
                             name="a4e", bufs=2)

            def b16(t):
                return t[:, :, :, None].broadcast_to([128, K2, 8, 16])

            nc.vector.tensor_tensor(a4e[:, 1], b16(g0), b16(wx), OP.mult)
            nc.vector.tensor_tensor(a4e[:, 0], b16(g0), a4e[:, 1],
                                    OP.subtract)
            nc.vector.tensor_tensor(a4e[:, 3], b16(g1), b16(wx), OP.mult)
            nc.vector.tensor_tensor(a4e[:, 2], b16(g1), a4e[:, 3],
                                    OP.subtract)

            return a4e, idxg

        def emit_main(q, coefs, mid=None):
            a4e, idxg = coefs
            midc = None
            # 3-5. gather / diag-scale-transpose+sum (PE) / GEMM
            po = [psO.tile([128, 512], F32, tag=f"po{i}", name=f"po{i}")
                  for i in range(4)]
            for k in range(K2):
                gbuf = gpool.tile([128, BPQ, 1024], F16, tag="gbuf")
                nc.gpsimd.dma_gather(
                    gbuf[:],
                    quad[:],
                    idxg[:, k, :, :],
                    num_idxs=BPQ * 128,
                    num_idxs_reg=BPQ * 128,
                    elem_size=1024,
                    single_packet=False,
                    queue_num=(q * K2 + k) % 4,
                )
                # The per-(pixel,k) corner coefficients ride the PE: the
                # transpose matmuls use rhs = diag(a_j) per (corner, block)
                # instead of the identity, so scale + 4-corner sum + pixel
                # transpose all happen in the PSUM accumulation.  DVE only
                # builds the diag tiles (ident x per-partition coeff).
                dg = bpool.tile([128, 4, BPQ, 128], F16, tag="diag",
                                name="diag", bufs=3)
                for j in range(4):
                    nc.vector.tensor_tensor(
                        dg[:, j].rearrange('p b (r c) -> p b r c', r=8),
                        identblk[:].rearrange('p b (r c) -> p b r c', r=8),
                        a4e[:, j, k, :, None, :].broadcast_to(
                            [128, BPQ, 8, 16]),
                        OP.mult)
                if k == 2 and mid is not None:
                    midc = mid()

                for j2 in range(2):
                    for ct in range(2):
                        tp = psB.tile([128, 512], F32, tag="stage",
                                      name="tp_b")
                        for r in range(4):
                            bl = j2 * 4 + r
                            for j in range(4):
                                nc.tensor.matmul(
                                    tp[:, r * 128:(r + 1) * 128],
                                    gbuf[:, bl,
                                         j * 256 + ct * 128:
                                         j * 256 + ct * 128 + 128],
                                    dg[:, j, bl, :],
                                    start=(j == 0),
                                    stop=(j == 3),
                                )
                        rhs16 = rhsp.tile([128, 512], F16, tag="rhs",
                                          name="rhs")
                        nc.scalar.copy(rhs16[:], tp[:])
                        for ot in range(2):
                            widx = (k * 2 + ct) * 2 + ot
                            nc.tensor.matmul(
                                po[j2 * 2 + ot][:],
                                wm_sb[:, widx, :],
                                rhs16[:],
                                start=(k == 0 and ct == 0),
                                stop=(k == 8 and ct == 1),
                            )
            for j2 in range(2):
                og = q * 2 + j2
                for ot in range(2):
                    o_sb = outp.tile([128, 4, 128], F32, tag="osb")
                    nc.scalar.copy(o_sb[:], po[j2 * 2 + ot][:])
                    nc.sync.dma_start(out[ot, :, og * 4:(og + 1) * 4, :], o_sb[:])
            return midc

        # om(q+1) fills PE slack before/between mains; the coeff pipeline
        # of q+1 is emitted mid-main(q) (after k=2's diag) so its indices
        # are ready well before the boundary and the q+1 gathers never
        # wait on DVE.
        t0 = emit_prep_om(0)
        c0 = emit_prep_coeff(0, t0)
        t1 = emit_prep_om(1)
        c1 = emit_main(0, c0, mid=lambda: emit_prep_coeff(1, t1))
        t2 = emit_prep_om(2)
        c2 = emit_main(1, c1, mid=lambda: emit_prep_coeff(2, t2))
        t3 = emit_prep_om(3)
        c3 = emit_main(2, c2, mid=lambda: emit_prep_coeff(3, t3))
        emit_main(3, c3)

    nc.finalize()
    return nc


# ----------------------------------------------------------------------------
# host-side data prep
# ----------------------------------------------------------------------------

def build_in_maps(x, w_conv, b_conv, w_off, b_off, w_mask, b_mask):
    x = np.ascontiguousarray(x, np.float32)

    # quad image per batch: quad[(y0+2)*131+(x0+2), (j,c)] fp16, 2-pad so
    # clamped out-of-range corners read guaranteed zeros
    quads = []
    for b in range(B):
        xp = np.zeros((H + 4, W + 4, CIN), np.float32)
        xp[2:-2, 2:-2] = x[b].transpose(1, 2, 0)
        q = np.empty((QD, QD, 4, CIN), np.float16)
        q[:, :, 0] = xp[0:QD, 0:QD]
        q[:, :, 1] = xp[0:QD, 1:QD + 1]
        q[:, :, 2] = xp[1:QD + 1, 0:QD]
        q[:, :, 3] = xp[1:QD + 1, 1:QD + 1]
        quads.append(np.ascontiguousarray(q.reshape(NQ, 1024)))

    # offset/mask weights, output channels reordered to [dy*9, dx*9, ml*9]
    wom = np.concatenate([w_off, w_mask], 0).reshape(27, CIN, K2)  # [o,c,k]
    perm = np.concatenate([np.arange(0, 18, 2), np.arange(1, 18, 2),
                           np.arange(18, 27)])
    womp = wom[perm]                                   # [27(dy,dx,ml), c, k]
    womt = np.zeros((18, 128, 32), np.float32)
    for t in range(18):
        k, ch = divmod(t, 2)
        womt[t, :, 0:27] = womp[:, ch * 128:(ch + 1) * 128, k].T
    bom = np.concatenate([b_off, b_mask]).astype(np.float32)[perm]
    bofft = np.zeros((32, 1), np.float32)
    bofft[0:27, 0] = bom

    # main weights [c, (k,ct,ot), o] fp16
    wc = w_conv.reshape(COUT, CIN, K2)
    wmaint = np.zeros((128, 36, 128), np.float16)
    for k in range(K2):
        for ct in range(2):
            for ot in range(2):
                widx = (k * 2 + ct) * 2 + ot
                wmaint[:, widx, :] = (
                    wc[ot * 128:(ot + 1) * 128, ct * 128:(ct + 1) * 128, k].T
                )

    ky = (np.arange(K2) // 3).astype(np.float32)
    kx = (np.arange(K2) % 3).astype(np.float32)
    bXc = np.zeros((128, K2, NBLK), np.float32)
    bXc[:] = (np.arange(128, dtype=np.float32)[:, None, None]
              + kx[None, :, None] - 1.5)

    in_maps = []
    for core in range(NCORE):
        b, slab = divmod(core, 4)
        h0 = slab * HS
        xsl = np.zeros((2, 128, 34, 130), np.float32)
        r_lo = max(0, h0 - 1)
        r_hi = min(H, h0 + HS + 1)
        xsl[:, :, (r_lo - (h0 - 1)):(r_hi - (h0 - 1)), 1:129] = (
            x[b].reshape(2, 128, H, W)[:, :, r_lo:r_hi, :]
        )
        bYc = np.zeros((128, K2, NBLK), np.float32)
        bYc[:] = ((h0 + np.arange(NBLK, dtype=np.float32))[None, None, :]
                  + ky[None, :, None] - 1.5)
        in_maps.append({
"xslab": xsl,
            "quad": quads[b],
            "womt": womt,
            "wmaint": wmaint,
            "baseY": bYc,
            "baseX": np.ascontiguousarray(bXc),
            "bofft": bofft,
        })
    return in_maps


_PROGRAM = None
LAST_EXEC_NS = None
LAST_RESULTS = None


def kernel(x, w_conv, b_conv, w_off, b_off, w_mask, b_mask):
    global _PROGRAM, LAST_EXEC_NS, LAST_RESULTS
    in_maps = build_in_maps(x, w_conv, b_conv, w_off, b_off, w_mask, b_mask)
    if _PROGRAM is None:
        _PROGRAM = build_program()
    nc = _PROGRAM
    trace = bool(os.environ.get("DCN_TRACE"))
    res = run_bass_kernel_spmd(nc, in_maps, core_ids=list(range(NCORE)),
                               trace=trace)
    LAST_EXEC_NS = res.exec_time_ns
    LAST_RESULTS = res
    out = np.zeros((B, COUT, H, W), np.float32)
    for core in range(NCORE):
        b, slab = divmod(core, 4)
        h0 = slab * HS
        oc = res.results[core]["out"]  # [2, 128, 32, 128]
        out[b, 0:128, h0:h0 + HS, :] = oc[0]
        out[b, 128:256, h0:h0 + HS, :] = oc[1]
    # b_conv is zeros in the reference setup, but add anyway for correctness
    out += np.asarray(b_conv, np.float32)[None, :, None, None]
    return out

